# revision 1
# baseline (speedup 1.0000x reference)
"""HGCN forward on 8 TRN2 NeuronCores.

Strategy (graph/data parallel, per sharding hint):
- Nodes padded to 100352 = 8*12544 and sharded across cores (12544/core).
- Per-node math (hyperboloid linear/exp/log maps) in node-major [128,128]
  SBUF tiles; dense weights replicated; weight matmuls via PE transpose.
- hyp_agg: edges sorted by destination tile; per 128-edge chunk, gather
  xt[src] rows with indirect DMA from a replicated xt table (built each
  layer by AllGather of per-core shards), build a one-hot*weight matrix
  with a fused tensor_scalar (is_equal, mult) against an iota constant,
  and accumulate dst-tile aggregates on the TensorEngine in PSUM.
"""
import sys, types
import numpy as np

sys.path.insert(0, "/opt/trn_rl_repo")

# NTFF profile hook shim (antenv.axon_hooks is absent in this image).
if "antenv.axon_hooks" not in sys.modules:
    _m = types.ModuleType("antenv.axon_hooks")
    _hh = [None]
    _m.set_axon_ntff_profile_hook = lambda h: _hh.__setitem__(0, h)
    _m.get_axon_ntff_profile_hook = lambda: _hh[0]
    sys.modules["antenv.axon_hooks"] = _m
    try:
        from trn_agent_boot.trn_boot import _ntff_profile_via_ctypes
        _m.set_axon_ntff_profile_hook(_ntff_profile_via_ctypes("/opt/axon/libaxon_pjrt.so"))
    except Exception:
        pass

import concourse.bass as bass
import concourse.tile as tile
from concourse import bacc, mybir
import concourse.bass_utils as _bu
_bu.upload_artifacts = lambda d: "local://skipped"
from concourse.bass_utils import run_bass_kernel_spmd
from contextlib import ExitStack

F = np.float32
EPS = 1e-7
MIN = 1e-15
NC = 8
P = 128
DT = mybir.dt.float32


def _host_ub(b, c):
    # u_b = logmap0(proj(expmap0(proj_tan0(b), c), c), c), faithful f32.
    K = F(1.0 / c)
    sK = F(np.sqrt(K))
    y = b[1:].astype(F)
    yn = max(np.sqrt((y * y).sum(dtype=F)), F(MIN))
    th = min(yn / sK, F(15.0))
    sh = F(np.sinh(th))
    ch = F(np.cosh(th))
    hb_s = sK * sh * y / yn
    hb0 = F(np.sqrt(max(K + (hb_s * hb_s).sum(dtype=F), F(EPS))))
    thh = max(hb0 / sK, F(1.0 + EPS))
    ac = F(np.log(thh + np.sqrt(thh * thh - 1)))
    ybn = max(F(np.sqrt((hb_s * hb_s).sum(dtype=F))), F(MIN))
    u_s = sK * ac * hb_s / ybn
    out = np.zeros(b.shape[0], F)
    out[1:] = u_s
    return out


def _build(T, Kc, NPAD, out_d=64):
    """One SPMD program for all 8 cores. T node-tiles/core, Kc chunks/tile."""
    S = T * P
    nc = bacc.Bacc("TRN2", target_bir_lowering=False, debug=False, num_devices=NC)

    xpT = nc.dram_tensor("xpT", [T, P, P], DT, kind="ExternalInput")
    idx_d = nc.dram_tensor("idx", [T, P, Kc], mybir.dt.int32, kind="ExternalInput")
    meta_d = nc.dram_tensor("meta", [T, P, 2 * Kc], DT, kind="ExternalInput")
    consts = nc.dram_tensor("consts", [P, 896], DT, kind="ExternalInput")
    out_d_t = nc.dram_tensor("out", [S, out_d], DT, kind="ExternalOutput")

    xt1_sh = nc.dram_tensor("xt1_sh", [S, P], DT)
    xt1_full = nc.dram_tensor("xt1_full", [NPAD, P], DT, addr_space="Shared")
    xt2_sh = nc.dram_tensor("xt2_sh", [S, P], DT)
    xt2_full = nc.dram_tensor("xt2_full", [NPAD, P], DT, addr_space="Shared")

    sK = [F(np.sqrt(3.0)), F(np.sqrt(2.0)), F(1.0)]
    A = mybir.AluOpType

    with tile.TileContext(nc) as tc, ExitStack() as ctx:
        cp = ctx.enter_context(tc.tile_pool(name="consts", bufs=1))
        xpp = ctx.enter_context(tc.tile_pool(name="xp", bufs=3))
        gp = ctx.enter_context(tc.tile_pool(name="gath", bufs=2))
        mp = ctx.enter_context(tc.tile_pool(name="meta", bufs=2))
        ip = ctx.enter_context(tc.tile_pool(name="idx", bufs=2))
        wk = ctx.enter_context(tc.tile_pool(name="work", bufs=3))
        sc = ctx.enter_context(tc.tile_pool(name="scal", bufs=3))
        mtp = ctx.enter_context(tc.tile_pool(name="mt", bufs=3))
        pag = ctx.enter_context(tc.tile_pool(name="pag", bufs=2, space="PSUM"))
        pmv = ctx.enter_context(tc.tile_pool(name="pmv", bufs=2, space="PSUM"))
        ptr = ctx.enter_context(tc.tile_pool(name="ptr", bufs=2, space="PSUM"))

        ct = cp.tile([P, 896], DT)
        nc.sync.dma_start(out=ct[:], in_=consts[:])
        W1T = ct[:, 0:128]
        W2T = ct[:, 128:256]
        WlT = ct[:, 256:320]
        UB1 = ct[:, 320:448]
        UB2 = ct[:, 448:576]
        UBL = ct[:, 576:640]
        IDN = ct[:, 640:768]
        IOTA = ct[:, 768:896]

        _nsn = [0]

        def ns():  # fresh scalar tile
            _nsn[0] = (_nsn[0] + 1) % 40
            nm = "s" + str(_nsn[0])
            return sc.tile([P, 1], DT, tag=nm, name=nm)

        def expmap_mobius(mv_ps, UB, k, D):
            """expmap0+proj then mobius_add(+u_b)+proj at curvature index k.
            mv_ps: PSUM [P, D] (col0 junk). Returns (L tile [P,D], ln2 [P,1], L0 [P,1])."""
            sk = float(sK[k]); ik = 1.0 / sk; K = sk * sk
            scr = wk.tile([P, D], DT, tag="scr", name="scr")
            mn2 = ns()
            nc.scalar.activation(scr[:, 1:D], mv_ps[:, 1:D], mybir.ActivationFunctionType.Square, accum_out=mn2[:])
            mnr = ns(); nc.scalar.sqrt(mnr[:], mn2[:])
            mnc = ns(); nc.vector.tensor_scalar(mnc[:], mnr[:], MIN, None, A.max)
            thc = ns(); nc.vector.tensor_scalar(thc[:], mnc[:], ik, 15.0, A.mult, A.min)
            ea = ns(); nc.scalar.activation(ea[:], thc[:], mybir.ActivationFunctionType.Exp)
            eb = ns(); nc.scalar.activation(eb[:], thc[:], mybir.ActivationFunctionType.Exp, scale=-1.0)
            sh2 = ns(); nc.vector.tensor_tensor(sh2[:], ea[:], eb[:], A.subtract)
            ch2 = ns(); nc.vector.tensor_tensor(ch2[:], ea[:], eb[:], A.add)
            rmn = ns(); nc.vector.reciprocal(rmn[:], mnc[:])
            g1 = ns(); nc.vector.tensor_scalar(g1[:], sh2[:], rmn[:, :1], 0.5 * sk, A.mult, A.mult)
            x0v = ns(); nc.vector.tensor_scalar(x0v[:], ch2[:], 0.5 * sk, None, A.mult)
            r1 = wk.tile([P, D], DT, tag="r1", name="r1")
            nc.scalar.activation(r1[:, :D], mv_ps[:, :D], mybir.ActivationFunctionType.Copy, scale=g1[:, :1])
            nc.scalar.copy(r1[:, 0:1], x0v[:])
            yn = ns(); nc.vector.tensor_scalar(yn[:], g1[:], mnc[:, :1], MIN, A.mult, A.max)
            nc.vector.tensor_tensor(scr[:, 1:D], r1[:, 1:D], UB[:, 1:D], A.mult)
            d1 = ns(); nc.vector.tensor_reduce(d1[:], scr[:, 1:D], mybir.AxisListType.X, A.add)
            ryn = ns(); nc.vector.reciprocal(ryn[:], yn[:])
            alpha = ns(); nc.vector.tensor_scalar(alpha[:], d1[:], ryn[:, :1], ik, A.mult, A.mult)
            skx = ns(); nc.vector.tensor_scalar(skx[:], x0v[:], sk, -1.0, A.subtract, A.mult)
            t2 = ns(); nc.vector.tensor_tensor(t2[:], alpha[:], skx[:], A.mult)
            scal1 = ns(); nc.vector.tensor_tensor(scal1[:], t2[:], ryn[:], A.mult)
            t3 = wk.tile([P, D], DT, tag="t3", name="t3")
            nc.vector.tensor_scalar(t3[:, :D], r1[:, :D], scal1[:, :1], None, A.mult)
            res = wk.tile([P, D], DT, tag="res", name="res")
            nc.vector.tensor_tensor(res[:, :D], UB[:, :D], t3[:, :D], A.subtract)
            nc.vector.tensor_tensor(scr[:, 1:D], r1[:, 1:D], res[:, 1:D], A.mult)
            ux = ns(); nc.vector.tensor_reduce(ux[:], scr[:, 1:D], mybir.AxisListType.X, A.add)
            rx0 = ns(); nc.vector.reciprocal(rx0[:], x0v[:])
            v0 = ns(); nc.vector.tensor_tensor(v0[:], ux[:], rx0[:], A.mult)
            nc.scalar.copy(res[:, 0:1], v0[:])  # res is now v
            mdp = ns()
            nc.scalar.activation(scr[:, 1:D], res[:, 1:D], mybir.ActivationFunctionType.Square, accum_out=mdp[:])
            v0q = ns(); nc.vector.tensor_tensor(v0q[:], v0[:], v0[:], A.mult)
            md = ns(); nc.vector.tensor_tensor(md[:], mdp[:], v0q[:], A.subtract)
            mdc = ns(); nc.vector.tensor_scalar(mdc[:], md[:], EPS, None, A.max)
            nur = ns(); nc.scalar.sqrt(nur[:], mdc[:])
            th2 = ns(); nc.vector.tensor_scalar(th2[:], nur[:], 1e6, ik, A.min, A.mult)
            th2m = ns(); nc.vector.tensor_scalar(th2m[:], th2[:], MIN, None, A.max)
            th2c = ns(); nc.vector.tensor_scalar(th2c[:], th2m[:], 15.0, None, A.min)
            ea2 = ns(); nc.scalar.activation(ea2[:], th2c[:], mybir.ActivationFunctionType.Exp)
            eb2 = ns(); nc.scalar.activation(eb2[:], th2c[:], mybir.ActivationFunctionType.Exp, scale=-1.0)
            sh22 = ns(); nc.vector.tensor_tensor(sh22[:], ea2[:], eb2[:], A.subtract)
            ch22 = ns(); nc.vector.tensor_tensor(ch22[:], ea2[:], eb2[:], A.add)
            rt2 = ns(); nc.vector.reciprocal(rt2[:], th2m[:])
            s2 = ns(); nc.vector.tensor_scalar(s2[:], sh22[:], rt2[:, :1], 0.5, A.mult, A.mult)
            t4 = wk.tile([P, D], DT, tag="t4", name="t4")
            nc.vector.tensor_scalar(t4[:, :D], r1[:, :D], ch22[:, :1], 0.5, A.mult, A.mult)
            t5 = wk.tile([P, D], DT, tag="t5", name="t5")
            nc.scalar.activation(t5[:, :D], res[:, :D], mybir.ActivationFunctionType.Copy, scale=s2[:, :1])
            L = wk.tile([P, D], DT, tag="L", name="L")
            nc.vector.tensor_tensor(L[:, :D], t4[:, :D], t5[:, :D], A.add)
            ln2 = ns()
            nc.scalar.activation(scr[:, 1:D], L[:, 1:D], mybir.ActivationFunctionType.Square, accum_out=ln2[:])
            lnk = ns(); nc.vector.tensor_scalar(lnk[:], ln2[:], float(K), None, A.add)
            L0 = ns(); nc.scalar.sqrt(L0[:], lnk[:])
            nc.scalar.copy(L[:, 0:1], L0[:])
            return L, ln2, L0

        def logmap_xt(L, ln2, L0, k):
            sk = float(sK[k]); ik = 1.0 / sk
            ynr = ns(); nc.scalar.sqrt(ynr[:], ln2[:])
            ync = ns(); nc.vector.tensor_scalar(ync[:], ynr[:], MIN, None, A.max)
            thL = ns(); nc.vector.tensor_scalar(thL[:], L0[:], ik, 1.0 + EPS, A.mult, A.max)
            tq = ns(); nc.vector.tensor_tensor(tq[:], thL[:], thL[:], A.mult)
            tqm = ns(); nc.vector.tensor_scalar(tqm[:], tq[:], -1.0, None, A.add)
            sq = ns(); nc.scalar.sqrt(sq[:], tqm[:])
            ai = ns(); nc.vector.tensor_tensor(ai[:], thL[:], sq[:], A.add)
            ac = ns(); nc.scalar.activation(ac[:], ai[:], mybir.ActivationFunctionType.Ln)
            ry = ns(); nc.vector.reciprocal(ry[:], ync[:])
            fL = ns(); nc.vector.tensor_scalar(fL[:], ac[:], ry[:, :1], sk, A.mult, A.mult)
            xt = wk.tile([P, P], DT, tag="xt", name="xt")
            nc.scalar.activation(xt[:], L[:], mybir.ActivationFunctionType.Copy, scale=fL[:, :1])
            return xt

        def agg_tile(t, table):
            idx_t = ip.tile([P, Kc], mybir.dt.int32, name="idx_t")
            nc.sync.dma_start(out=idx_t[:], in_=idx_d[t])
            met = mp.tile([P, 2 * Kc], DT, name="met")
            nc.sync.dma_start(out=met[:], in_=meta_d[t])
            G = gp.tile([P, Kc * P], DT, tag="G", name="G")
            for kk in range(Kc):
                nc.gpsimd.indirect_dma_start(
                    out=G[:, kk * P:(kk + 1) * P], out_offset=None,
                    in_=table[:],
                    in_offset=bass.IndirectOffsetOnAxis(ap=idx_t[:, kk:kk + 1], axis=0),
                )
            agg = pag.tile([P, P], DT, space="PSUM", name="aggp")
            for kk in range(Kc):
                Mt = mtp.tile([P, P], DT, tag="Mt", name="Mt")
                nc.vector.tensor_scalar(Mt[:], IOTA[:], met[:, kk:kk + 1], met[:, Kc + kk:Kc + kk + 1],
                                        A.is_equal, A.mult)
                nc.tensor.matmul(agg[:], lhsT=Mt[:], rhs=G[:, kk * P:(kk + 1) * P],
                                 start=(kk == 0), stop=(kk == Kc - 1))
            return agg

        def post_agg(agg, kin, kout):
            ski, iki = float(sK[kin]), 1.0 / float(sK[kin])
            sko, iko = float(sK[kout]), 1.0 / float(sK[kout])
            scr2 = wk.tile([P, P], DT, tag="scr2", name="scr2")
            an2 = ns()
            nc.scalar.activation(scr2[:, 1:P], agg[:, 1:P], mybir.ActivationFunctionType.Square, accum_out=an2[:])
            anr = ns(); nc.scalar.sqrt(anr[:], an2[:])
            anc = ns(); nc.vector.tensor_scalar(anc[:], anr[:], MIN, None, A.max)
            th3 = ns(); nc.vector.tensor_scalar(th3[:], anc[:], iki, 15.0, A.mult, A.min)
            ran = ns(); nc.vector.reciprocal(ran[:], anc[:])
            h3 = ns(); nc.vector.tensor_scalar(h3[:], th3[:], ran[:, :1], ski, A.mult, A.mult)
            xt2 = wk.tile([P, P], DT, tag="xt2", name="xt2")
            nc.vector.tensor_scalar(xt2[:], agg[:], h3[:, :1], 0.0, A.mult, A.max)
            y42 = ns()
            nc.scalar.activation(scr2[:, 1:P], xt2[:, 1:P], mybir.ActivationFunctionType.Square, accum_out=y42[:])
            y4r = ns(); nc.scalar.sqrt(y4r[:], y42[:])
            y4c = ns(); nc.vector.tensor_scalar(y4c[:], y4r[:], MIN, None, A.max)
            th4 = ns(); nc.vector.tensor_scalar(th4[:], y4c[:], iko, 15.0, A.mult, A.min)
            r4 = ns(); nc.vector.reciprocal(r4[:], y4c[:])
            m5 = ns(); nc.vector.tensor_scalar(m5[:], th4[:], r4[:, :1], sko, A.mult, A.mult)
            lg = wk.tile([P, P], DT, tag="lg", name="lg")
            nc.scalar.activation(lg[:], xt2[:], mybir.ActivationFunctionType.Copy, scale=m5[:, :1])
            return lg

        def lin_mm(lg, WT, D):
            trp = ptr.tile([P, P], DT, space="PSUM", name="trp")
            nc.tensor.transpose(trp[:], lg[:], IDN[:])
            lgT = wk.tile([P, P], DT, tag="lgT", name="lgT")
            nc.vector.tensor_copy(lgT[:], trp[:])
            mv = pmv.tile([P, D], DT, space="PSUM", tag="mv", name="mvp")
            nc.tensor.matmul(mv[:], lhsT=lgT[:], rhs=WT[:, :D], start=True, stop=True)
            return mv

        # ---- Phase A ----
        for t in range(T):
            xt_in = xpp.tile([P, P], DT)
            nc.sync.dma_start(out=xt_in[:], in_=xpT[t])
            mv = pmv.tile([P, P], DT, space="PSUM", tag="mv")
            nc.tensor.matmul(mv[:], lhsT=xt_in[:], rhs=W1T[:], start=True, stop=True)
            L, ln2, L0 = expmap_mobius(mv, UB1, 0, P)
            xt = logmap_xt(L, ln2, L0, 0)
            nc.sync.dma_start(out=xt1_sh[t * P:(t + 1) * P, :], in_=xt[:])
        nc.gpsimd.collective_compute("AllGather", A.bypass, replica_groups=[list(range(NC))],
                                     ins=[xt1_sh[:]], outs=[xt1_full[:]])
        # ---- Phase B ----
        for t in range(T):
            agg = agg_tile(t, xt1_full)
            lg2 = post_agg(agg, 0, 1)
            mv2 = lin_mm(lg2, W2T, P)
            L2, ln2b, L0b = expmap_mobius(mv2, UB2, 1, P)
            xt2t = logmap_xt(L2, ln2b, L0b, 1)
            nc.sync.dma_start(out=xt2_sh[t * P:(t + 1) * P, :], in_=xt2t[:])
        nc.gpsimd.collective_compute("AllGather", A.bypass, replica_groups=[list(range(NC))],
                                     ins=[xt2_sh[:]], outs=[xt2_full[:]])
        # ---- Phase C ----
        for t in range(T):
            agg = agg_tile(t, xt2_full)
            lg3 = post_agg(agg, 1, 2)
            mv3 = lin_mm(lg3, WlT, out_d)
            Lf, _, _ = expmap_mobius(mv3, UBL, 2, out_d)
            nc.sync.dma_start(out=out_d_t[t * P:(t + 1) * P, :], in_=Lf[:])

    nc.compile()
    return nc


def _prep(x, edge_index, edge_weight, W1, b1, W2, b2, Wl, bl, NPAD):
    N = x.shape[0]
    S = NPAD // NC
    T = S // P
    GT = NPAD // P
    src = edge_index[0].astype(np.int64)
    dst = edge_index[1].astype(np.int64)
    w = edge_weight.astype(F)
    order = np.argsort(dst, kind="stable")
    srcs, dsts, ws = src[order], dst[order], w[order]
    gt = dsts >> 7
    cnt = np.bincount(gt, minlength=GT)
    Kc = max(1, int(np.ceil(cnt.max() / P)))
    CAP = Kc * P
    starts = np.zeros(GT, np.int64)
    starts[1:] = np.cumsum(cnt)[:-1]
    pos = np.arange(len(srcs)) - starts[gt]
    pad_src = np.zeros((GT, CAP), np.int32)
    pad_rel = np.zeros((GT, CAP), F)
    pad_w = np.zeros((GT, CAP), F)
    pad_src[gt, pos] = srcs
    pad_rel[gt, pos] = (dsts - (gt << 7)).astype(F)
    pad_w[gt, pos] = ws

    # layouts per core: idx [T,P,Kc] with idx[t,p,k]=edge (t,k*128+p); meta [T,P,2Kc]
    idx_all = pad_src.reshape(GT, Kc, P).transpose(0, 2, 1)          # [GT,P,Kc]
    rel_all = pad_rel.reshape(GT, Kc, P).transpose(0, 2, 1)
    w_all = pad_w.reshape(GT, Kc, P).transpose(0, 2, 1)
    meta_all = np.concatenate([rel_all, w_all], axis=2)              # [GT,P,2Kc]

    xp = np.zeros((NPAD, P), F)
    xp[:N, 1:] = x
    Tc = T
    xpT = xp.reshape(NPAD // P, P, P).transpose(0, 2, 1)             # [GT,P,P] transposed tiles

    def ZW(Wm):
        We = Wm.astype(F).copy()
        We[:, 0] = 0
        return np.ascontiguousarray(We.T)

    ub1 = _host_ub(b1.astype(F), 1.0 / 3.0)
    ub2 = _host_ub(b2.astype(F), 0.5)
    ubl = _host_ub(bl.astype(F), 1.0)
    consts = np.zeros((P, 896), F)
    consts[:, 0:128] = ZW(W1)
    consts[:, 128:256] = ZW(W2)
    consts[:, 256:320] = ZW(Wl)
    consts[:, 320:448] = np.tile(ub1, (P, 1))
    consts[:, 448:576] = np.tile(ub2, (P, 1))
    consts[:, 576:640] = np.tile(ubl, (P, 1))
    consts[:, 640:768] = np.eye(P, dtype=F)
    consts[:, 768:896] = np.tile(np.arange(P, dtype=F), (P, 1))

    in_maps = []
    for c in range(NC):
        in_maps.append({
            "xpT": np.ascontiguousarray(xpT[c * Tc:(c + 1) * Tc]),
            "idx": np.ascontiguousarray(idx_all[c * Tc:(c + 1) * Tc]),
            "meta": np.ascontiguousarray(meta_all[c * Tc:(c + 1) * Tc]),
            "consts": consts,
        })
    return in_maps, T, Kc


_CACHE = {}


def kernel(x, edge_index, edge_weight, W1, b1, W2, b2, Wl, bl, trace=False):
    N = x.shape[0]
    NPAD = ((N + NC * P - 1) // (NC * P)) * NC * P
    in_maps, T, Kc = _prep(x, edge_index, edge_weight, W1, b1, W2, b2, Wl, bl, NPAD)
    key = (T, Kc, NPAD)
    if key not in _CACHE:
        _CACHE[key] = _build(T, Kc, NPAD, 64)
    nc = _CACHE[key]
    r = run_bass_kernel_spmd(nc, in_maps, list(range(NC)), trace=trace)
    out = np.concatenate([r.results[c]["out"] for c in range(NC)], axis=0)[:N]
    kernel.last_exec_ns = r.exec_time_ns
    return out.astype(np.float32)


kernel.last_exec_ns = None



# revision 4
# speedup vs baseline: 1.4568x; 1.4568x over previous
"""HGCN forward on 8 TRN2 NeuronCores — optimized v2.

Strategy vs baseline:
- Algebraic collapse: each HypLinear+mobius_add+logmap0 layer reduces to
  xt = alpha[node] * mv + beta[node] * u_b, where mv = lg @ Wz.T (one bf16
  matmul with an extra column Wz.T@u_b giving the <mv,u_b> dot for free) and
  alpha/beta come from a per-node scalar chain fed by 2 reductions.
  logmap0(proj(expmap0(.))) pairs collapse to norm-clip identities.
- bf16 matmuls/tables (fp32 matmul = 2 HW passes; bf16 = 1 + fast wt load).
- Scalar chains batched across 49-tile groups as [128,49] ops (kills ACT
  table-reload storm + per-op overhead).
- Gathers via dma_gather: one SWDGE call per (7-tile group x table quarter)
  instead of one indirect DMA per 128 edges; int16 indices relative to a
  quarter of the node table. Chunk geometry uniform across cores (SPMD).
- xt tables in bf16: halves gather + AllGather traffic.
"""
import os, sys, types
import numpy as np

os.environ.setdefault("NEURON_RT_RESET_CORES", "1")

sys.path.insert(0, "/opt/trn_rl_repo")

if "antenv.axon_hooks" not in sys.modules:
    _m = types.ModuleType("antenv.axon_hooks")
    _hh = [None]
    _m.set_axon_ntff_profile_hook = lambda h: _hh.__setitem__(0, h)
    _m.get_axon_ntff_profile_hook = lambda: _hh[0]
    sys.modules["antenv.axon_hooks"] = _m
    try:
        from trn_agent_boot.trn_boot import _ntff_profile_via_ctypes
        _m.set_axon_ntff_profile_hook(_ntff_profile_via_ctypes("/opt/axon/libaxon_pjrt.so"))
    except Exception:
        pass

import ml_dtypes
import concourse.bass as bass
import concourse.tile as tile
from concourse import bacc, mybir
import concourse.bass_utils as _bu
_bu.upload_artifacts = lambda d: "local://skipped"
from concourse.bass_utils import run_bass_kernel_spmd
from contextlib import ExitStack

F = np.float32
BFNP = ml_dtypes.bfloat16
EPS = 1e-7
MIN = 1e-15
NC = 8
P = 128
NQ = 4          # node-table quarters (int16 index range)
GG = 7          # tiles per gather-group
DT = mybir.dt.float32
BF = mybir.dt.bfloat16
I16 = mybir.dt.int16
sK = [float(np.sqrt(3.0)), float(np.sqrt(2.0)), 1.0]
A = None  # set in _build


def _host_ub(b, c):
    K = F(1.0 / c)
    sk = F(np.sqrt(K))
    y = b[1:].astype(F)
    yn = max(np.sqrt((y * y).sum(dtype=F)), F(MIN))
    th = min(yn / sk, F(15.0))
    sh = F(np.sinh(th)); ch = F(np.cosh(th))
    hb_s = sk * sh * y / yn
    hb0 = F(np.sqrt(max(K + (hb_s * hb_s).sum(dtype=F), F(EPS))))
    thh = max(hb0 / sk, F(1.0 + EPS))
    ac = F(np.log(thh + np.sqrt(thh * thh - 1)))
    ybn = max(F(np.sqrt((hb_s * hb_s).sum(dtype=F))), F(MIN))
    u_s = sk * ac * hb_s / ybn
    out = np.zeros(b.shape[0], F)
    out[1:] = u_s
    return out


def _build(T, NPAD, plan, out_w=64):
    global A
    S = T * P
    G2 = T // 2  # chain-group width (tiles)
    assert T % 2 == 0 and T % GG == 0
    NPADQ = NPAD // NQ
    nc = bacc.Bacc("TRN2", target_bir_lowering=False, debug=False, num_devices=NC)
    A = mybir.AluOpType
    AFT = mybir.ActivationFunctionType

    ICOLS = plan["icols"]
    TOTCH = plan["totch"]
    MAXCHQ = plan["maxchq"]     # max chunks per (gg, q) call
    MAXNCH = plan["maxnch"]     # max chunks per gg
    ggs = plan["ggs"]

    xpT_d = nc.dram_tensor("xpT", [T, P, P], BF, kind="ExternalInput")
    idx_d = nc.dram_tensor("idx16", [P, ICOLS], I16, kind="ExternalInput")
    meta_d = nc.dram_tensor("meta", [P, 2 * TOTCH], DT, kind="ExternalInput")
    ctB_d = nc.dram_tensor("ctB", [P, 771], BF, kind="ExternalInput")
    ctF_d = nc.dram_tensor("ctF", [P, 131], DT, kind="ExternalInput")
    out_d = nc.dram_tensor("out", [S, out_w], DT, kind="ExternalOutput")

    # Collective tensors are declared fp32 (half the columns, same bytes):
    # the AllGather firmware path is only proven on fp32; producers/consumers
    # bitcast to bf16 views.
    xt1_sh = nc.dram_tensor("xt1_sh", [S, P // 2], DT)
    xt1_full = nc.dram_tensor("xt1_full", [NPAD, P // 2], DT, addr_space="Shared")
    xt2_sh = nc.dram_tensor("xt2_sh", [S, P // 2], DT)
    xt2_full = nc.dram_tensor("xt2_full", [NPAD, P // 2], DT, addr_space="Shared")

    with tile.TileContext(nc) as tc, ExitStack() as ctx:
        cp = ctx.enter_context(tc.tile_pool(name="consts", bufs=1))
        xpp = ctx.enter_context(tc.tile_pool(name="xp", bufs=3))
        gp = ctx.enter_context(tc.tile_pool(name="gath", bufs=2))
        ip = ctx.enter_context(tc.tile_pool(name="idx", bufs=2))
        mp = ctx.enter_context(tc.tile_pool(name="meta", bufs=2))
        mtp = ctx.enter_context(tc.tile_pool(name="mt", bufs=4))
        wk = ctx.enter_context(tc.tile_pool(name="work", bufs=3))
        grp = ctx.enter_context(tc.tile_pool(name="grp", bufs=2))
        cbp = ctx.enter_context(tc.tile_pool(name="cb", bufs=2))
        cpl = ctx.enter_context(tc.tile_pool(name="chain", bufs=2))
        pag = ctx.enter_context(tc.tile_pool(name="pag", bufs=2, space="PSUM"))
        pmv = ctx.enter_context(tc.tile_pool(name="pmv", bufs=2, space="PSUM"))
        ptr = ctx.enter_context(tc.tile_pool(name="ptr", bufs=2, space="PSUM"))

        ctB = cp.tile([P, 771], BF)
        nc.sync.dma_start(out=ctB[:], in_=ctB_d[:])
        ctF = cp.tile([P, 131], DT)
        nc.sync.dma_start(out=ctF[:], in_=ctF_d[:])
        W1a = ctB[:, 0:129]
        W2a = ctB[:, 129:258]
        Wla = ctB[:, 258:323]
        UB1 = ctB[:, 323:451]
        UB2 = ctB[:, 451:579]
        UBL = ctB[:, 579:643]
        IDN = ctB[:, 643:771]
        IOTA = ctF[:, 0:128]
        SuuA = [ctF[:, 128:129], ctF[:, 129:130], ctF[:, 130:131]]

        def _mkops(prefix):
            """Tag-scoped chain op helpers; tags reset per chain instance so
            storage is reused (pool bufs=2 covers adjacent instances)."""
            n = [0]

            def ct_():
                n[0] += 1
                nm = "%s%d" % (prefix, n[0])
                return cpl.tile([P, G2], DT, tag=nm, name=nm)

            def ts(in_, s1, s2, o1, o2=None, out=None):
                t = out if out is not None else ct_()
                if o2 is None:
                    nc.vector.tensor_scalar(t[:], in_, s1, s2, o1)
                else:
                    nc.vector.tensor_scalar(t[:], in_, s1, s2, o1, o2)
                return t

            def tt(in0, in1, op, out=None):
                t = out if out is not None else ct_()
                nc.vector.tensor_tensor(t[:], in0, in1, op)
                return t

            def sqr(in_):
                t = ct_()
                nc.scalar.sqrt(t[:], in_)
                return t

            def rcp(in_):
                t = ct_()
                nc.vector.reciprocal(t[:], in_)
                return t

            def ex(in_, scale=1.0):
                t = ct_()
                nc.scalar.activation(t[:], in_, AFT.Exp, scale=scale)
                return t

            def ln_(in_):
                t = ct_()
                nc.scalar.activation(t[:], in_, AFT.Ln)
                return t

            return ts, tt, sqr, rcp, ex, ln_

        def clip_chain(n2, k):
            """min(1, 15*sK[k] / max(sqrt(n2), MIN)) -- [P,G2]."""
            ts, tt, sqr, rcp, ex, ln_ = _mkops("cl")
            r = sqr(n2[:])
            rc = ts(r[:], MIN, None, A.max)
            ra = rcp(rc[:])
            return ts(ra[:], 15.0 * sK[k], 1.0, A.mult, A.min)

        def chain(mn2_t, d1_t, k, Suu, final, m5=None):
            """Per-node scalar chain on [P,G2]. Returns (alpha, beta, L0)."""
            ts, tt, sqr, rcp, ex, ln_ = _mkops("ch")
            sk = sK[k]; ik = 1.0 / sk; K = sk * sk
            if m5 is not None:
                m5sq = tt(m5[:], m5[:], A.mult)
                mn2 = tt(mn2_t[:], m5sq[:], A.mult)
                d1p = tt(d1_t[:], m5[:], A.mult)
            else:
                mn2, d1p = mn2_t, d1_t
            mnr = sqr(mn2[:])
            mnc = ts(mnr[:], MIN, None, A.max)
            thc = ts(mnc[:], ik, 15.0, A.mult, A.min)
            ea = ex(thc[:]); eb = ex(thc[:], scale=-1.0)
            sh2 = tt(ea[:], eb[:], A.subtract)
            ch2 = tt(ea[:], eb[:], A.add)
            rmn = rcp(mnc[:])
            g1a = tt(sh2[:], rmn[:], A.mult)
            g1 = ts(g1a[:], 0.5 * sk, None, A.mult)
            x0v = ts(ch2[:], 0.5 * sk, None, A.mult)
            d1g = tt(d1p[:], g1[:], A.mult)
            yna = tt(g1[:], mnc[:], A.mult)
            yn = ts(yna[:], MIN, None, A.max)
            ryn = rcp(yn[:])
            ala = tt(d1g[:], ryn[:], A.mult)
            alp = ts(ala[:], ik, None, A.mult)
            skx = ts(x0v[:], sk, -1.0, A.subtract, A.mult)
            t2 = tt(alp[:], skx[:], A.mult)
            scal1 = tt(t2[:], ryn[:], A.mult)
            ynq = tt(yn[:], yn[:], A.mult)
            sq_ynq = tt(scal1[:], ynq[:], A.mult)
            ux = tt(d1g[:], sq_ynq[:], A.subtract)
            rx0 = rcp(x0v[:])
            v0 = tt(ux[:], rx0[:], A.mult)
            a1 = tt(scal1[:], d1g[:], A.mult)
            a3 = tt(scal1[:], sq_ynq[:], A.mult)
            a1b = ts(a1[:], 2.0, None, A.mult)
            a4 = tt(a3[:], a1b[:], A.subtract)
            mdp = ts(a4[:], Suu, None, A.add)
            v0q = tt(v0[:], v0[:], A.mult)
            md = tt(mdp[:], v0q[:], A.subtract)
            mdc = ts(md[:], EPS, None, A.max)
            nur = sqr(mdc[:])
            th2 = ts(nur[:], 1e6, ik, A.min, A.mult)
            th2m = ts(th2[:], MIN, None, A.max)
            th2c = ts(th2m[:], 15.0, None, A.min)
            ea2 = ex(th2c[:]); eb2 = ex(th2c[:], scale=-1.0)
            sh22 = tt(ea2[:], eb2[:], A.subtract)
            ch22 = tt(ea2[:], eb2[:], A.add)
            rt2 = rcp(th2m[:])
            s2a = tt(sh22[:], rt2[:], A.mult)
            s2 = ts(s2a[:], 0.5, None, A.mult)
            a5 = tt(s2[:], scal1[:], A.mult)
            ch2h = ts(ch22[:], 0.5, None, A.mult)
            a_ = tt(ch2h[:], a5[:], A.subtract)
            ag = tt(a_[:], g1[:], A.mult)
            agq = tt(ag[:], ag[:], A.mult)
            b2t = tt(agq[:], mn2[:], A.mult)
            b3t = tt(ag[:], s2[:], A.mult)
            b4 = tt(b3t[:], d1p[:], A.mult)
            b4b = ts(b4[:], 2.0, None, A.mult)
            b5 = tt(s2[:], s2[:], A.mult)
            b6 = ts(b5[:], Suu, None, A.mult)
            l_a = tt(b2t[:], b4b[:], A.add)
            ln2 = tt(l_a[:], b6[:], A.add)
            lnk = ts(ln2[:], K, None, A.add)
            L0 = sqr(lnk[:])
            if final:
                alpha = tt(ag[:], m5[:], A.mult) if m5 is not None else ag
                return alpha, s2, L0
            ynr = sqr(ln2[:])
            ync = ts(ynr[:], MIN, None, A.max)
            thL = ts(L0[:], ik, 1.0 + EPS, A.mult, A.max)
            tq = tt(thL[:], thL[:], A.mult)
            tqm = ts(tq[:], -1.0, None, A.add)
            sqq = sqr(tqm[:])
            ai = tt(thL[:], sqq[:], A.add)
            acl = ln_(ai[:])
            ry = rcp(ync[:])
            fLa = tt(acl[:], ry[:], A.mult)
            fL = ts(fLa[:], sk, None, A.mult)
            alpha = tt(fL[:], ag[:], A.mult)
            if m5 is not None:
                alpha = tt(alpha[:], m5[:], A.mult)
            beta = tt(fL[:], s2[:], A.mult)
            return alpha, beta, L0

        # ---------------- phase emitters ----------------

        def phase(l):
            """l=0: input linear; l=1: agg@C0 + linear@C1; l=2: agg@C1 + final linear@C2."""
            has_agg = l > 0
            final = l == 2
            Wsl = [W1a, W2a, Wla][l]
            Dw = 129 if l < 2 else 65
            UBt = [UB1, UB2, UBL][l]
            Uw = 128 if l < 2 else 64
            tbl = (xt1_full if l == 1 else xt2_full)[:].bitcast(BF)
            sink = [xt1_sh, xt2_sh, None][l]
            k_agg = l - 1
            Suu = SuuA[l][:, 0:1]

            groups = [(0, G2), (G2, T)]
            st = [dict() for _ in groups]

            def s1(gi):
                g0, g1 = groups[gi]
                d = st[gi]
                if has_agg:
                    d["an2"] = cbp.tile([P, G2], DT, tag="an2", name="an2")
                    d["aggS"] = grp.tile([P, G2 * P], BF, tag="aggS", name="aggS")
                else:
                    d["mn2"] = cbp.tile([P, G2], DT, tag="mn2", name="mn2")
                    d["d1"] = cbp.tile([P, G2], DT, tag="d1", name="d1")
                    d["mvS"] = grp.tile([P, G2 * P], BF, tag="mvS", name="mvS")
                if not has_agg:
                    for t in range(g0, g1):
                        i = t - g0
                        xin = xpp.tile([P, P], BF, tag="xin", name="xin")
                        nc.sync.dma_start(out=xin[:], in_=xpT_d[t])
                        mv = pmv.tile([P, Dw], DT, space="PSUM", tag="mvB", name="mvB")
                        nc.tensor.matmul(mv[:], lhsT=xin[:], rhs=Wsl[:, :Dw], start=True, stop=True)
                        scr = wk.tile([P, P - 1], DT, tag="scr", name="scr")
                        acc = wk.tile([P, 1], DT, tag="acc", name="acc")
                        nc.scalar.activation(scr[:, :127], mv[:, 1:128], AFT.Square,
                                             accum_out=acc[:])
                        nc.vector.tensor_scalar(d["mn2"][:, i:i + 1], acc[:], 0.0, None, A.add)
                        nc.vector.tensor_scalar(d["d1"][:, i:i + 1], mv[:, 128:129], 0.0, None, A.add)
                        nc.vector.tensor_scalar(d["mvS"][:, i * P:(i + 1) * P], mv[:, 0:P], 0.0, None, A.add)
                    return
                for ggi in range(g0 // GG, g1 // GG):
                    info = ggs[ggi]
                    nch = info["nch"]
                    met = mp.tile([P, 2 * MAXNCH], DT, tag="met", name="met")
                    nc.sync.dma_start(out=met[:, :2 * nch],
                                      in_=meta_d[:, info["mcol"]:info["mcol"] + 2 * nch])
                    Gq = [None] * NQ
                    for q in range(NQ):
                        cap = info["caps"][q]
                        if cap == 0:
                            continue
                        it = ip.tile([P, MAXCHQ * 8], I16, tag="iq%d" % q, name="iq%d" % q)
                        nc.sync.dma_start(out=it[:, :cap // 16],
                                          in_=idx_d[:, info["icol"][q]:info["icol"][q] + cap // 16])
                        g = gp.tile([P, MAXCHQ, P], BF, tag="Gq%d" % q, name="Gq%d" % q)
                        nc.gpsimd.dma_gather(
                            out_ap=g[:, :cap // P, :],
                            in_ap=tbl[q * NPADQ:(q + 1) * NPADQ, :],
                            idxs_ap=it[:, :cap // 16],
                            num_idxs=cap,
                            num_idxs_reg=cap,
                            elem_size=P,
                            single_packet=False,
                        )
                        Gq[q] = g
                    for trel, chunks in enumerate(info["tiles"]):
                        t = ggi * GG + trel
                        i = t - g0
                        agg = pag.tile([P, P], DT, space="PSUM", tag="agg", name="agg")
                        ncq = len(chunks)
                        for jj, (q, pos, mj) in enumerate(chunks):
                            Mt = mtp.tile([P, P], BF, tag="Mt", name="Mt")
                            nc.vector.tensor_scalar(
                                Mt[:], IOTA[:], met[:, 2 * mj:2 * mj + 1],
                                met[:, 2 * mj + 1:2 * mj + 2], A.is_equal, A.mult)
                            nc.tensor.matmul(agg[:], lhsT=Mt[:], rhs=Gq[q][:, pos, :],
                                             start=(jj == 0), stop=(jj == ncq - 1))
                        scr = wk.tile([P, P - 1], DT, tag="scr", name="scr")
                        acc = wk.tile([P, 1], DT, tag="acc", name="acc")
                        nc.scalar.activation(scr[:], agg[:, 1:P], AFT.Square,
                                             accum_out=acc[:])
                        nc.vector.tensor_scalar(d["an2"][:, i:i + 1], acc[:], 0.0, None, A.add)
                        nc.vector.tensor_scalar(d["aggS"][:, i * P:(i + 1) * P], agg[:], 0.0, None, A.add)

            def ch1(gi):
                d = st[gi]
                d["h3"] = clip_chain(d["an2"], k_agg)

            def s2(gi):
                g0, g1 = groups[gi]
                d = st[gi]
                d["mn2"] = cbp.tile([P, G2], DT, tag="mn2", name="mn2")
                d["d1"] = cbp.tile([P, G2], DT, tag="d1", name="d1")
                d["y42"] = cbp.tile([P, G2], DT, tag="y42", name="y42")
                if final:
                    d["mvS"] = grp.tile([P, G2 * out_w], DT, tag="mvSC", name="mvSC")
                else:
                    d["mvS"] = grp.tile([P, G2 * P], BF, tag="mvS", name="mvS")
                h3 = d["h3"]
                for t in range(g0, g1):
                    i = t - g0
                    xt2 = wk.tile([P, P], BF, tag="xt2", name="xt2")
                    nc.vector.tensor_scalar(xt2[:], d["aggS"][:, i * P:(i + 1) * P],
                                            h3[:, i:i + 1], 0.0, A.mult, A.max)
                    sq2 = wk.tile([P, P - 1], DT, tag="sq2", name="sq2")
                    acc2 = wk.tile([P, 1], DT, tag="acc2", name="acc2")
                    nc.scalar.activation(sq2[:], xt2[:, 1:P], AFT.Square,
                                         accum_out=acc2[:])
                    nc.vector.tensor_scalar(d["y42"][:, i:i + 1], acc2[:], 0.0, None, A.add)
                    trp = ptr.tile([P, P], BF, space="PSUM", tag="trp", name="trp")
                    nc.tensor.transpose(trp[:], xt2[:], IDN[:])
                    xt2T = wk.tile([P, P], BF, tag="xt2T", name="xt2T")
                    nc.vector.tensor_copy(xt2T[:], trp[:])
                    mv = pmv.tile([P, Dw], DT, space="PSUM", tag="mvB", name="mvB")
                    nc.tensor.matmul(mv[:], lhsT=xt2T[:], rhs=Wsl[:, :Dw], start=True, stop=True)
                    scr = wk.tile([P, P - 1], DT, tag="scr", name="scr")
                    acc3 = wk.tile([P, 1], DT, tag="acc3", name="acc3")
                    nc.scalar.activation(scr[:, :Dw - 2], mv[:, 1:Dw - 1], AFT.Square,
                                         accum_out=acc3[:])
                    nc.vector.tensor_scalar(d["mn2"][:, i:i + 1], acc3[:], 0.0, None, A.add)
                    nc.vector.tensor_scalar(d["d1"][:, i:i + 1], mv[:, Dw - 1:Dw], 0.0, None, A.add)
                    if final:
                        nc.vector.tensor_scalar(d["mvS"][:, i * out_w:(i + 1) * out_w],
                                                mv[:, 0:out_w], 0.0, None, A.add)
                    else:
                        nc.vector.tensor_scalar(d["mvS"][:, i * P:(i + 1) * P],
                                                mv[:, 0:P], 0.0, None, A.add)

            def ch2(gi):
                d = st[gi]
                m5 = None
                if has_agg:
                    m5 = clip_chain(d["y42"], l)
                d["alpha"], d["beta"], d["L0"] = chain(
                    d["mn2"], d["d1"], l, Suu, final, m5=m5)

            def s3(gi):
                g0, g1 = groups[gi]
                d = st[gi]
                al, be, L0 = d["alpha"], d["beta"], d["L0"]
                for t in range(g0, g1):
                    i = t - g0
                    if final:
                        o1 = wk.tile([P, out_w], DT, tag="o1", name="o1")
                        nc.vector.tensor_scalar(o1[:], d["mvS"][:, i * out_w:(i + 1) * out_w],
                                                al[:, i:i + 1], None, A.mult)
                        o2 = wk.tile([P, out_w], DT, tag="o2", name="o2")
                        nc.vector.tensor_scalar(o2[:], UBt[:, :Uw], be[:, i:i + 1], None, A.mult)
                        ot = wk.tile([P, out_w], DT, tag="o3", name="o3")
                        nc.vector.tensor_tensor(ot[:], o1[:], o2[:], A.add)
                        nc.vector.tensor_scalar(ot[:, 0:1], L0[:, i:i + 1], 0.0, None, A.add)
                        nc.sync.dma_start(out=out_d[t * P:(t + 1) * P, :], in_=ot[:])
                    else:
                        f1 = wk.tile([P, P], BF, tag="f1", name="f1")
                        nc.vector.tensor_scalar(f1[:], d["mvS"][:, i * P:(i + 1) * P],
                                                al[:, i:i + 1], None, A.mult)
                        f2 = wk.tile([P, P], BF, tag="f2", name="f2")
                        nc.vector.tensor_scalar(f2[:], UBt[:, :Uw], be[:, i:i + 1], None, A.mult)
                        f3 = wk.tile([P, P], BF, tag="f3", name="f3")
                        nc.vector.tensor_tensor(f3[:], f1[:], f2[:], A.add)
                        nc.sync.dma_start(out=sink[t * P:(t + 1) * P, :], in_=f3[:].bitcast(DT))

            if has_agg:
                s1(0); s1(1)
                ch1(0); s2(0); ch2(0); s3(0)
                ch1(1); s2(1); ch2(1); s3(1)
            else:
                s1(0); s1(1)
                ch2(0); s3(0)
                ch2(1); s3(1)

        # ---------------- program ----------------
        phase(0)
        nc.gpsimd.collective_compute("AllGather", mybir.AluOpType.bypass,
                                     replica_groups=[list(range(NC))],
                                     ins=[xt1_sh[:]], outs=[xt1_full[:]])
        phase(1)
        nc.gpsimd.collective_compute("AllGather", mybir.AluOpType.bypass,
                                     replica_groups=[list(range(NC))],
                                     ins=[xt2_sh[:]], outs=[xt2_full[:]])
        phase(2)

    nc.compile()
    return nc


def _prep(x, edge_index, edge_weight, W1, b1, W2, b2, Wl, bl, NPAD):
    N = x.shape[0]
    S = NPAD // NC
    T = S // P
    GT = NPAD // P
    NPADQ = NPAD // NQ
    NGG = T // GG
    src = edge_index[0].astype(np.int64)
    dst = edge_index[1].astype(np.int64)
    w = edge_weight.astype(F)

    # bin edges by (dst tile, src quarter)
    gt = dst >> 7
    qe = src // NPADQ
    key = gt * NQ + qe
    order = np.argsort(key, kind="stable")
    s2_, d2_, w2_, k2_ = src[order], dst[order], w[order], key[order]
    cnt = np.bincount(k2_, minlength=GT * NQ).reshape(NC, T, NQ)

    # uniform chunk geometry across cores (SPMD shares one program)
    chunks_tq = np.ceil(cnt.max(axis=0) / P).astype(np.int64)     # [T, NQ]
    caps_tq = chunks_tq * P

    # padded layout per core, ordered (gg -> q -> t): bin (t,q) at bin_start[t,q]
    bin_start = np.zeros((T, NQ), np.int64)
    gg_q_start = np.zeros((NGG, NQ), np.int64)
    off = 0
    for ggi in range(NGG):
        for q in range(NQ):
            gg_q_start[ggi, q] = off
            for trel in range(GG):
                t = ggi * GG + trel
                bin_start[t, q] = off
                off += caps_tq[t, q]
    TOTCAP = int(off)
    assert TOTCAP % 16 == 0

    # scatter edges into the padded layout (per core)
    bin_of_edge = (k2_ % (T * NQ))       # (t*NQ + q) within core
    t_of_edge = bin_of_edge // NQ
    q_of_edge = bin_of_edge % NQ
    core_of_edge = k2_ // (T * NQ)
    # position within bin
    pos_in_bin = np.arange(len(k2_)) - np.concatenate(
        [[0], np.cumsum(np.bincount(k2_, minlength=GT * NQ))])[k2_]
    tgt = bin_start[t_of_edge, q_of_edge] + pos_in_bin

    idxrel_pad = np.zeros((NC, TOTCAP), np.int16)
    rel_pad = np.zeros((NC, TOTCAP), F)
    w_pad = np.zeros((NC, TOTCAP), F)
    idxrel_pad[core_of_edge, tgt] = (s2_ - q_of_edge * NPADQ).astype(np.int16)
    rel_pad[core_of_edge, tgt] = (d2_ & 127).astype(F)
    w_pad[core_of_edge, tgt] = w2_

    # plan + per-core idx16 / meta arrays
    ggs = []
    TOTCH = int(chunks_tq.sum())
    idx16 = np.zeros((NC, P, TOTCAP // 16), np.int16)
    meta = np.zeros((NC, P, 2 * TOTCH), F)
    mcol = 0
    maxchq = 0
    for ggi in range(NGG):
        caps = []
        icol = []
        for q in range(NQ):
            cap = int(caps_tq[ggi * GG:(ggi + 1) * GG, q].sum())
            caps.append(cap)
            icol.append(int(gg_q_start[ggi, q] // 16))
            if cap:
                maxchq = max(maxchq, cap // P)
                sl = slice(int(gg_q_start[ggi, q]), int(gg_q_start[ggi, q]) + cap)
                # wrapped int16 layout: flat i -> [i%16 (replicated), i//16]
                wv = idxrel_pad[:, sl].reshape(NC, cap // 16, 16).transpose(0, 2, 1)
                idx16[:, :, gg_q_start[ggi, q] // 16:(gg_q_start[ggi, q] + cap) // 16] = (
                    np.tile(wv, (1, 8, 1)))
        tiles = []
        mj = 0
        gg_mcol = mcol
        for trel in range(GG):
            t = ggi * GG + trel
            tlist = []
            for q in range(NQ):
                nchq = int(chunks_tq[t, q])
                posbase = int((bin_start[t, q] - gg_q_start[ggi, q]) // P)
                for c in range(nchq):
                    sl = slice(int(bin_start[t, q]) + c * P, int(bin_start[t, q]) + (c + 1) * P)
                    meta[:, :, 2 * (gg_mcol + mj)] = rel_pad[:, sl]
                    meta[:, :, 2 * (gg_mcol + mj) + 1] = w_pad[:, sl]
                    tlist.append((q, posbase + c, mj))
                    mj += 1
            tiles.append(tlist)
        ggs.append({"caps": caps, "icol": icol, "mcol": 2 * gg_mcol,
                    "nch": mj, "tiles": tiles})
        mcol = gg_mcol + mj
    maxnch = max(g["nch"] for g in ggs)

    plan = {
        "icols": TOTCAP // 16,
        "totch": TOTCH,
        "maxchq": maxchq,
        "maxnch": maxnch,
        "ggs": ggs,
    }

    # encode + norm-clip on host: lg1 = [0,x] * min(1, 15*sqrt(3)/max(|x|,MIN))
    xf = x.astype(F)
    xn = np.sqrt((xf * xf).sum(axis=1, dtype=F))
    s = np.minimum(F(15.0 * np.sqrt(3.0)) / np.maximum(xn, F(MIN)), F(1.0))
    xp = np.zeros((NPAD, P), F)
    xp[:N, 1:] = xf * s[:, None]
    xpT = np.ascontiguousarray(xp.reshape(GT, P, P).transpose(0, 2, 1)).astype(BFNP)

    def ZW(Wm):
        We = Wm.astype(F).copy()
        We[:, 0] = 0
        return We

    W1z, W2z, Wlz = ZW(W1), ZW(W2), ZW(Wl)
    ub1 = _host_ub(b1.astype(F), 1.0 / 3.0)
    ub2 = _host_ub(b2.astype(F), 0.5)
    ubl = _host_ub(bl.astype(F), 1.0)

    ctB = np.zeros((P, 771), F)
    ctB[:, 0:128] = W1z.T
    ctB[:, 128] = W1z.T @ ub1
    ctB[:, 129:257] = W2z.T
    ctB[:, 257] = W2z.T @ ub2
    ctB[:, 258:322] = Wlz.T
    ctB[:, 322] = Wlz.T @ ubl
    ctB[:, 323:451] = np.tile(ub1, (P, 1))
    ctB[:, 451:579] = np.tile(ub2, (P, 1))
    ctB[:, 579:643] = np.tile(ubl[:64], (P, 1))
    ctB[:, 643:771] = np.eye(P, dtype=F)
    ctB = ctB.astype(BFNP)

    ctF = np.zeros((P, 131), F)
    ctF[:, 0:128] = np.tile(np.arange(P, dtype=F), (P, 1))
    ctF[:, 128] = (ub1 * ub1).sum(dtype=F)
    ctF[:, 129] = (ub2 * ub2).sum(dtype=F)
    ctF[:, 130] = (ubl * ubl).sum(dtype=F)

    in_maps = []
    for c in range(NC):
        in_maps.append({
            "xpT": np.ascontiguousarray(xpT[c * T:(c + 1) * T]),
            "idx16": np.ascontiguousarray(idx16[c]),
            "meta": np.ascontiguousarray(meta[c]),
            "ctB": ctB,
            "ctF": ctF,
        })
    return in_maps, T, plan


_CACHE = {}


def kernel(x, edge_index, edge_weight, W1, b1, W2, b2, Wl, bl, trace=False):
    N = x.shape[0]
    NPAD = ((N + NC * P - 1) // (NC * P)) * NC * P
    in_maps, T, plan = _prep(x, edge_index, edge_weight, W1, b1, W2, b2, Wl, bl, NPAD)
    key = (T, NPAD, tuple(tuple(g["caps"]) for g in plan["ggs"]))
    if key not in _CACHE:
        _CACHE[key] = _build(T, NPAD, plan, 64)
    nc = _CACHE[key]
    r = run_bass_kernel_spmd(nc, in_maps, list(range(NC)), trace=trace)
    out = np.concatenate([r.results[c]["out"] for c in range(NC)], axis=0)[:N]
    kernel.last_exec_ns = r.exec_time_ns
    return out.astype(np.float32)


kernel.last_exec_ns = None
